# revision 9
# baseline (speedup 1.0000x reference)
"""Chamfer distance loss on 8 Trainium2 NeuronCores.

Problem: template/source [4, 4096, 3] f32 -> scalar loss
  d[b,n,m] = ||t_n - s_m||^2 ; mean_n(min_m d) + mean_m(min_n d), mean over b.

Strategy (data-parallel over batch x template-half, 2 cores per batch):
  Each core handles one batch's full source set (4096 pts) against one half of
  the template set (2048 pts). Distances come from a single matmul in NEGATED
  split-bf16 form (K=24 rows reproduce negd = 2 t.s - |t|^2 - |s|^2 to
  near-fp32 accuracy at full PE rate), so every reduction is a MAX.

  v4 pipeline (110 -> ~89us on the harness metric):
  Per template block i (16 blocks of 128 points):
    - 8 matmuls -> 2 psum tiles [128, 2048] f32 (2-tile rotation).
    - ACT casts both tiles -> ct_i [128, 4096] f16. ACT is the loop pacer
      (~63us of ACTIVATE); PE (~57us) and DVE (~83us busy) overlap under it.
    - DVE d01 (row max): fold1 ct_i -> pair staging; per pair of blocks a
      4-level fold chain + small TENSOR_REDUCE -> negd01 slots. All TT ops
      are f16 SBUF->SBUF, 2x DVE mode, sized >=256 elems.
    - DVE d10 (col max): P_j = max(ct_2j, ct_2j+1), then a running
      acc = max(acc, P_j). 15 ops of [128, 4096] at 2x.
  Tail: at the last block the d10 chain runs FIRST (acc quarters -> 8 PE
  transposes -> psum f16 -> TENSOR_REDUCE -> negd10 slots per quarter), then
  the deferred d01 fold1+closure overlaps the transpose/reduce pipeline. Engine-op constraints found the
  hard way: TensorTensor/TensorReduce only lower on DVE (walrus rejects them
  on ACT/Pool), TT reads at most one PSUM operand, DMA cannot touch PSUM and
  its compute mode is add-only, matmul PSUM output must be f32.
"""

import numpy as np
import ml_dtypes

import concourse.bass as bass
import concourse.bass_utils as bass_utils
import concourse.tile as tile
from concourse import mybir
from concourse.bass_utils import run_bass_kernel_spmd
from concourse.vector_clock import ScopedClock

B, N, M = 4, 4096, 4096
HALF = N // 2  # template half per core: 2048
N_CORES = 8
TBLOCKS = HALF // 128  # 16 template blocks
SFREE = M // 2  # source half width: 2048
K = 24

F32 = mybir.dt.float32
F16 = mybir.dt.float16
BF16 = mybir.dt.bfloat16
MAX = mybir.AluOpType.max

_MAX_DRAIN_WAITS = 1

# pairs whose running-max op runs on the Activation engine (ACT has slack;
# DVE is the pacer). Empty tuple = all on DVE.
ACT_ACC_PAIRS = ()


class _ChunkedDrainTileContext(tile.TileContext):
    """The walrus build used by the axon/PJRT path rejects instructions with
    more than a couple of sync waits; Tile's exit drain attaches one wait per
    live logical processor. Split them across sequential drains."""

    def _drain_and_barrier(self, tick_clock, wait_clock):
        # Stock Tile emits drain + two all-engine barriers around semaphore
        # clears (~9us of measured tail). The kernel PREAMBLE already clears
        # semaphore ranges 150..255 on every execution, so end-of-kernel
        # clears are redundant for re-runs; the only load-bearing waits are
        # the DMA-queue completion sems (output data must land before the
        # program is considered done). Keep just those, on the sync engine.
        drain_inst = self.nc.sync.drain()
        wait_clock.add_sem_waits(
            drain_inst.ins, ScopedClock({None: tick_clock.global_clock})
        )
        si = drain_inst.ins.sync_info
        waits = list(si.on_wait) if si is not None and si.on_wait else []
        dma_waits = [w for w in waits if w.ant_name and "DMA" in w.ant_name]
        keep = dma_waits if dma_waits else waits
        drain_inst.ins.sync_info = mybir.SyncInfo(
            on_wait=keep[:_MAX_DRAIN_WAITS],
            on_update=list(si.on_update or []) if si else [],
        )
        rest = keep[_MAX_DRAIN_WAITS:]
        while rest:
            d = self.nc.sync.drain()
            d.ins.sync_info = mybir.SyncInfo(
                on_wait=rest[:_MAX_DRAIN_WAITS], on_update=[]
            )
            rest = rest[_MAX_DRAIN_WAITS:]

        assert self.sems is not None
        popped = self.nc._tile_sem_poison_stack.pop()
        assert popped is self._sem_poison


def _split_multi_waits(nc: bass.Bass, max_waits: int = 1) -> int:
    """This walrus build rejects instructions carrying several sync waits.
    Hoist excess waits onto NoOps inserted before the offender on the same
    engine — same-engine program order preserves the semantics."""
    n = 0
    for f in nc.m.functions:
        for bb in f.blocks:
            insts = list(bb.instructions)
            out = []
            changed = False
            for inst in insts:
                si = inst.sync_info
                if si is not None and si.on_wait and len(si.on_wait) > max_waits:
                    # sem_ge waits on the same semaphore are subsumed by the
                    # highest value: dedupe before splitting (the Tile drain
                    # attaches one wait per logical-processor tick, mostly on
                    # the same few engine sems).
                    best: dict = {}
                    order = []
                    passthrough = []
                    for w in si.on_wait:
                        if (
                            "ge" in str(w.wait_mode)
                            and w.wait_value is not None
                        ):
                            key = (str(w.sync_type), w.id)
                            cur = best.get(key)
                            if cur is None:
                                order.append(key)
                                best[key] = w
                            elif w.wait_value > cur.wait_value:
                                best[key] = w
                        else:
                            passthrough.append(w)
                    waits = passthrough + [best[k] for k in order]
                    if len(waits) <= max_waits:
                        inst.sync_info = mybir.SyncInfo(
                            on_wait=waits, on_update=list(si.on_update or [])
                        )
                        out.append(inst)
                        changed = True
                        continue
                    extra, keep = waits[:-max_waits], waits[-max_waits:]
                    while extra:
                        chunk, extra = extra[:max_waits], extra[max_waits:]
                        n += 1
                        out.append(
                            mybir.InstNoOp(
                                name=f"waitsplit-{n}",
                                engine=inst.engine,
                                sync_info=mybir.SyncInfo(on_wait=chunk, on_update=[]),
                            )
                        )
                    inst.sync_info = mybir.SyncInfo(
                        on_wait=keep, on_update=list(si.on_update or [])
                    )
                    changed = True
                out.append(inst)
            if changed:
                bb.instructions = out
    return n


def _tt_max(nc, eng, out_ap, in0, in1):
    """tensor_tensor(max) emitted on an arbitrary engine proxy."""
    return eng.add_instruction(
        mybir.InstTensorTensor(
            name=nc.get_next_instruction_name(),
            op=mybir.AluOpType.max,
            ins=[eng.lower_ap(in0), eng.lower_ap(in1)],
            outs=[eng.lower_ap(out_ap)],
        )
    )


def build_program() -> bass.Bass:
    nc = bass.Bass("TRN2", target_bir_lowering=True, debug=False)
    tmplA = nc.declare_dram_parameter("tmplA", [K, HALF], BF16, isOutput=False)
    srcA = nc.declare_dram_parameter("srcA", [K, M], BF16, isOutput=False)
    ident = nc.declare_dram_parameter("ident", [128, 128], F16, isOutput=False)
    negd01 = nc.declare_dram_parameter("negd01", [128, TBLOCKS], F32, isOutput=True)
    negd10 = nc.declare_dram_parameter("negd10", [128, M // 128], F32, isOutput=True)

    with _ChunkedDrainTileContext(nc) as tc:
        with (
            tc.tile_pool(name="inp", bufs=1) as inp,
            tc.tile_pool(name="psum", bufs=2, space="PSUM") as pp,
            tc.tile_pool(name="ct", bufs=4) as ctp,
            tc.tile_pool(name="s1", bufs=2) as s1p,
            tc.tile_pool(name="tree", bufs=2) as treep,
            tc.tile_pool(name="qs", bufs=2) as qsp,
            tc.tile_pool(name="outp", bufs=1) as outp,
        ):
            # input DMAs on separate queues; template block 0 lands first so
            # the first matmul can start immediately.
            tmpl_sb = inp.tile([K, HALF], BF16)
            nc.sync.dma_start(tmpl_sb[:, 0:128], tmplA[:, 0:128])
            nc.sync.dma_start(tmpl_sb[:, 128:HALF], tmplA[:, 128:HALF])
            src_sb = inp.tile([K, M], BF16)
            nc.gpsimd.dma_start(src_sb[:, 0:128], srcA[:, 0:128])
            nc.gpsimd.dma_start(src_sb[:, 128:1024], srcA[:, 128:1024])
            for piece in range(3):
                nc.gpsimd.dma_start(
                    src_sb[:, 1024 + piece * 1024 : 2048 + piece * 1024],
                    srcA[:, 1024 + piece * 1024 : 2048 + piece * 1024],
                )
            id_sb = inp.tile([128, 128], F16)
            nc.gpsimd.dma_start(id_sb[:], ident[:])

            d01sb = outp.tile([128, TBLOCKS], F32)
            d10sb = outp.tile([128, M // 128], F32)

            acc = None
            P_prev = None
            s1 = None
            for i in range(TBLOCKS):
                # ---- PE: 8 matmuls -> 2 psum tiles
                ps_h = []
                for h in range(2):
                    ps = pp.tile([128, SFREE], F32, tag="ps")
                    if i == 0 and h == 0:
                        # narrow first matmul: only needs src[0:128] + the
                        # first template block, so PE starts ~1us earlier
                        nc.tensor.matmul(
                            ps[:, 0:128], lhsT=tmpl_sb[:, 0:128],
                            rhs=src_sb[:, 0:128], start=True, stop=True,
                        )
                        nc.tensor.matmul(
                            ps[:, 128:512], lhsT=tmpl_sb[:, 0:128],
                            rhs=src_sb[:, 128:512], start=True, stop=True,
                        )
                        chunks = range(1, SFREE // 512)
                    else:
                        chunks = range(SFREE // 512)
                    for jj in chunks:
                        nc.tensor.matmul(
                            ps[:, bass.ts(jj, 512)],
                            lhsT=tmpl_sb[:, bass.ts(i, 128)],
                            rhs=src_sb[:, h * SFREE + jj * 512 : h * SFREE + (jj + 1) * 512],
                            start=True,
                            stop=True,
                        )
                    ps_h.append(ps)

                # ---- ACT: cast to f16 (loop pacer)
                ct = ctp.tile([128, M], F16, tag="ct")
                nc.scalar.copy(ct[:, 0:SFREE], ps_h[0][:])
                nc.scalar.copy(ct[:, SFREE:M], ps_h[1][:])

                # ---- DVE d01 fold1 into pair staging [128, 2, 2048]
                # (for the last block this is emitted after the d10 chain so
                # the transpose tail overlaps the remaining d01 closure)
                if i % 2 == 0:
                    s1 = s1p.tile([128, 2 * SFREE], F16, tag="s1")
                if i < TBLOCKS - 1:
                    nc.vector.tensor_tensor(
                        s1[:, bass.ts(i % 2, SFREE)], ct[:, 0:SFREE], ct[:, SFREE:M],
                        op=MAX,
                    )

                # ---- DVE d10: pair tree level + running acc
                if i % 2 == 0:
                    ct_even = ct
                else:
                    P = treep.tile([128, M], F16, tag="P")
                    nc.vector.tensor_tensor(
                        P[:].rearrange("p (b c) -> p b c", c=SFREE),
                        ct_even[:].rearrange("p (b c) -> p b c", c=SFREE),
                        ct[:].rearrange("p (b c) -> p b c", c=SFREE),
                        op=MAX,
                    )
                    if P_prev is None:
                        P_prev = P
                    elif i < TBLOCKS - 1:
                        acc_new = treep.tile([128, M], F16, tag="acc")
                        nc.vector.tensor_tensor(
                            acc_new[:].rearrange("p (b c) -> p b c", c=SFREE),
                            (acc if acc is not None else P_prev)[:].rearrange(
                                "p (b c) -> p b c", c=SFREE
                            ),
                            P[:].rearrange("p (b c) -> p b c", c=SFREE),
                            op=MAX,
                        )
                        acc = acc_new

                # ---- last block: d10 tail first (accq quarters + PE
                # transposes + TR), then the deferred fold1 + closure
                if i == TBLOCKS - 1:
                    accq = treep.tile([128, M], F16, tag="acc")
                    for qq in range(4):
                        csl = slice(qq * 1024, (qq + 1) * 1024)
                        nc.vector.tensor_tensor(
                            accq[:, csl], acc[:, csl], P[:, csl], op=MAX
                        )
                        psT = pp.tile([128, 1024], F16, tag="ps")
                        for t in range(8):
                            nc.tensor.transpose(
                                psT[:, bass.ts(t, 128)],
                                accq[:, qq * 1024 + t * 128 : qq * 1024 + (t + 1) * 128],
                                id_sb[:],
                            )
                        nc.vector.tensor_reduce(
                            d10sb[:, qq * 8 : (qq + 1) * 8],
                            psT[:].rearrange("p (t c) -> p t c", c=128),
                            axis=mybir.AxisListType.X,
                            op=MAX,
                        )
                        nc.sync.dma_start(
                            negd10[:, qq * 8 : (qq + 1) * 8],
                            d10sb[:, qq * 8 : (qq + 1) * 8],
                        )
                    nc.vector.tensor_tensor(
                        s1[:, bass.ts(i % 2, SFREE)], ct[:, 0:SFREE], ct[:, SFREE:M],
                        op=MAX,
                    )

                # ---- DVE d01 pair-group fold + TR -> negd01 slots
                if i % 2 == 1:
                    g = i // 2
                    sv = s1[:].rearrange("p (b c) -> p b c", c=SFREE)
                    g2 = qsp.tile([128, 2 * (SFREE // 2)], F16, tag="g2")
                    nc.vector.tensor_tensor(
                        g2[:].rearrange("p (b c) -> p b c", c=SFREE // 2),
                        sv[:, :, 0 : SFREE // 2],
                        sv[:, :, SFREE // 2 : SFREE],
                        op=MAX,
                    )
                    g3 = qsp.tile([128, 2 * (SFREE // 4)], F16, tag="g3")
                    nc.vector.tensor_tensor(
                        g3[:].rearrange("p (b c) -> p b c", c=SFREE // 4),
                        g2[:].rearrange("p (b c) -> p b c", c=SFREE // 2)[
                            :, :, 0 : SFREE // 4
                        ],
                        g2[:].rearrange("p (b c) -> p b c", c=SFREE // 2)[
                            :, :, SFREE // 4 : SFREE // 2
                        ],
                        op=MAX,
                    )
                    g4 = qsp.tile([128, 2 * (SFREE // 8)], F16, tag="g4")
                    nc.vector.tensor_tensor(
                        g4[:].rearrange("p (b c) -> p b c", c=SFREE // 8),
                        g3[:].rearrange("p (b c) -> p b c", c=SFREE // 4)[
                            :, :, 0 : SFREE // 8
                        ],
                        g3[:].rearrange("p (b c) -> p b c", c=SFREE // 4)[
                            :, :, SFREE // 8 : SFREE // 4
                        ],
                        op=MAX,
                    )
                    g5 = qsp.tile([128, 2 * (SFREE // 16)], F16, tag="g5")
                    nc.vector.tensor_tensor(
                        g5[:].rearrange("p (b c) -> p b c", c=SFREE // 16),
                        g4[:].rearrange("p (b c) -> p b c", c=SFREE // 8)[
                            :, :, 0 : SFREE // 16
                        ],
                        g4[:].rearrange("p (b c) -> p b c", c=SFREE // 8)[
                            :, :, SFREE // 16 : SFREE // 8
                        ],
                        op=MAX,
                    )
                    nc.vector.tensor_reduce(
                        d01sb[:, 2 * g : 2 * g + 2],
                        g5[:].rearrange("p (b c) -> p b c", c=SFREE // 16),
                        axis=mybir.AxisListType.X,
                        op=MAX,
                    )

            nc.sync.dma_start(negd01[:], d01sb[:])
    _split_multi_waits(nc)
    return nc


_PROGRAM = None


def get_program() -> bass.Bass:
    global _PROGRAM
    if _PROGRAM is None:
        _PROGRAM = build_program()
    return _PROGRAM


def _split3(x: np.ndarray):
    bf = ml_dtypes.bfloat16
    h1 = x.astype(bf).astype(np.float32)
    h2 = (x - h1).astype(bf).astype(np.float32)
    h3 = (x - h1 - h2).astype(bf).astype(np.float32)
    return h1, h2, h3


# cross-product levels kept: everything with combined magnitude >= ~2^-27
_PAIRS = [(0, 0), (0, 1), (1, 0), (0, 2), (1, 1), (2, 0)]


def make_in_maps(template: np.ndarray, source: np.ndarray) -> list[dict]:
    """Host-side prep: split-bf16 augmented K=24 representations, sharded per
    core. Core c -> batch c//2, template half c%2."""
    template = np.asarray(template, dtype=np.float32)
    source = np.asarray(source, dtype=np.float32)
    bf = ml_dtypes.bfloat16
    in_maps = []
    for c in range(N_CORES):
        b, hh = divmod(c, 2)
        t = template[b, hh * HALF : (hh + 1) * HALF]  # [HALF, 3]
        s = source[b]  # [M, 3]
        T = _split3(t)
        U = _split3((2.0 * s).astype(np.float32))
        nt = (t.astype(np.float64) ** 2).sum(-1).astype(np.float32)
        ns = (s.astype(np.float64) ** 2).sum(-1).astype(np.float32)
        NT = _split3(nt)
        NS = _split3(ns)
        ones_t = np.ones_like(nt)
        ones_s = np.ones_like(ns)
        a_rows, b_rows = [], []
        for cc in range(3):
            for (ii, jj) in _PAIRS:
                a_rows.append(T[ii][:, cc])
                b_rows.append(U[jj][:, cc])
        for kk in range(3):
            a_rows.append(-NT[kk])
            b_rows.append(ones_s)
            a_rows.append(-ones_t)
            b_rows.append(NS[kk])
        tmplA = np.stack(a_rows, 0).astype(bf)  # [K, HALF]
        srcA = np.stack(b_rows, 0).astype(bf)  # [K, M]
        in_maps.append(
            {
                "tmplA": np.ascontiguousarray(tmplA),
                "srcA": np.ascontiguousarray(srcA),
                "ident": np.eye(128, dtype=np.float16),
            }
        )
    return in_maps


def combine(results: list[dict]) -> np.ndarray:
    """Gather per-core partials into the scalar loss (float64 accumulation)."""
    per_batch = []
    for b in range(B):
        r0, r1 = results[2 * b], results[2 * b + 1]
        d01_parts = []
        for r in (r0, r1):
            nd01 = r["negd01"].astype(np.float64)  # [128, 16]
            # template index within half = i*128 + p -> transpose to [16,128]
            d01_parts.append(nd01.T.reshape(-1))
        d01 = -np.concatenate(d01_parts)  # [4096]
        # negd10[n_loc, h*16+t] for source index h*2048 + t*128 + n_loc
        nd10 = np.maximum(
            r0["negd10"].astype(np.float64), r1["negd10"].astype(np.float64)
        )
        d10 = -nd10.T.reshape(-1)  # [32,128] -> index t'*128+n_loc with t'=h*16+t
        per_batch.append(d01.mean() + d10.mean())
    return np.asarray(np.mean(per_batch), dtype=np.float32)


def _axon_reset():
    """Recover a wedged NeuronCore (NRT_EXEC_UNIT_UNRECOVERABLE) left by a
    previous crashed run, via the axon sidechannel."""
    try:
        import ctypes

        import jax

        jax.devices()
        lib = ctypes.CDLL("/opt/axon/libaxon_pjrt.so")
        lib.axon_reset.restype = ctypes.c_int64
        lib.axon_reset()
    except Exception:
        pass


def kernel(template: np.ndarray, source: np.ndarray) -> np.ndarray:
    nc = get_program()
    in_maps = make_in_maps(template, source)
    try:
        res = run_bass_kernel_spmd(nc, in_maps, list(range(N_CORES)))
    except Exception:
        _axon_reset()
        res = run_bass_kernel_spmd(nc, in_maps, list(range(N_CORES)))
    return combine(res.results)



# revision 10
# speedup vs baseline: 1.2062x; 1.2062x over previous
"""Chamfer distance loss on 8 Trainium2 NeuronCores.

Problem: template/source [4, 4096, 3] f32 -> scalar loss
  d[b,n,m] = ||t_n - s_m||^2 ; mean_n(min_m d) + mean_m(min_n d), mean over b.

Strategy (data-parallel over batch x template-half, 2 cores per batch):
  Each core handles one batch's full source set (4096 pts) against one half of
  the template set (2048 pts). Distances come from a single matmul in NEGATED
  split-bf16 form (K=24 rows reproduce negd = 2 t.s - |t|^2 - |s|^2 to
  near-fp32 accuracy at full PE rate), so every reduction is a MAX.

  v4 pipeline (110 -> ~89us on the harness metric):
  Per template block i (16 blocks of 128 points):
    - 8 matmuls -> 2 psum tiles [128, 2048] f32 (2-tile rotation).
    - ACT casts both tiles -> ct_i [128, 4096] f16. ACT is the loop pacer
      (~63us of ACTIVATE); PE (~57us) and DVE (~83us busy) overlap under it.
    - DVE d01 (row max): fold1 ct_i -> pair staging; per pair of blocks a
      4-level fold chain + small TENSOR_REDUCE -> negd01 slots. All TT ops
      are f16 SBUF->SBUF, 2x DVE mode, sized >=256 elems.
    - DVE d10 (col max): P_j = max(ct_2j, ct_2j+1), then a running
      acc = max(acc, P_j). 15 ops of [128, 4096] at 2x.
  Tail: at the last block the d10 chain runs FIRST (acc quarters -> 8 PE
  transposes -> psum f16 -> TENSOR_REDUCE -> negd10 slots per quarter), then
  the deferred d01 fold1+closure overlaps the transpose/reduce pipeline. Engine-op constraints found the
  hard way: TensorTensor/TensorReduce only lower on DVE (walrus rejects them
  on ACT/Pool), TT reads at most one PSUM operand, DMA cannot touch PSUM and
  its compute mode is add-only, matmul PSUM output must be f32.
"""

import numpy as np
import ml_dtypes

import concourse.bass as bass
import concourse.bass_utils as bass_utils
import concourse.tile as tile
from concourse import mybir
from concourse.bass_utils import run_bass_kernel_spmd
from concourse.vector_clock import ScopedClock

B, N, M = 4, 4096, 4096
HALF = N // 2  # template half per core: 2048
N_CORES = 8
TBLOCKS = HALF // 128  # 16 template blocks
SFREE = M // 2  # source half width: 2048
K = 24

F32 = mybir.dt.float32
F16 = mybir.dt.float16
BF16 = mybir.dt.bfloat16
MAX = mybir.AluOpType.max

_MAX_DRAIN_WAITS = 1

# pairs whose running-max op runs on the Activation engine (ACT has slack;
# DVE is the pacer). Empty tuple = all on DVE.
ACT_ACC_PAIRS = ()


class _ChunkedDrainTileContext(tile.TileContext):
    """The walrus build used by the axon/PJRT path rejects instructions with
    more than a couple of sync waits; Tile's exit drain attaches one wait per
    live logical processor. Split them across sequential drains."""

    def _drain_and_barrier(self, tick_clock, wait_clock):
        # Stock Tile emits drain + two all-engine barriers around semaphore
        # clears (~9us of measured tail). The kernel PREAMBLE already clears
        # semaphore ranges 150..255 on every execution, so end-of-kernel
        # clears are redundant for re-runs; the only load-bearing waits are
        # the DMA-queue completion sems (output data must land before the
        # program is considered done). Keep just those, on the sync engine.
        drain_inst = self.nc.sync.drain()
        wait_clock.add_sem_waits(
            drain_inst.ins, ScopedClock({None: tick_clock.global_clock})
        )
        si = drain_inst.ins.sync_info
        waits = list(si.on_wait) if si is not None and si.on_wait else []
        dma_waits = [w for w in waits if w.ant_name and "DMA" in w.ant_name]
        keep = dma_waits if dma_waits else waits
        drain_inst.ins.sync_info = mybir.SyncInfo(
            on_wait=keep[:_MAX_DRAIN_WAITS],
            on_update=list(si.on_update or []) if si else [],
        )
        rest = keep[_MAX_DRAIN_WAITS:]
        while rest:
            d = self.nc.sync.drain()
            d.ins.sync_info = mybir.SyncInfo(
                on_wait=rest[:_MAX_DRAIN_WAITS], on_update=[]
            )
            rest = rest[_MAX_DRAIN_WAITS:]

        assert self.sems is not None
        popped = self.nc._tile_sem_poison_stack.pop()
        assert popped is self._sem_poison


def _split_multi_waits(nc: bass.Bass, max_waits: int = 1) -> int:
    """This walrus build rejects instructions carrying several sync waits.
    Hoist excess waits onto NoOps inserted before the offender on the same
    engine — same-engine program order preserves the semantics."""
    n = 0
    for f in nc.m.functions:
        for bb in f.blocks:
            insts = list(bb.instructions)
            out = []
            changed = False
            for inst in insts:
                si = inst.sync_info
                if si is not None and si.on_wait and len(si.on_wait) > max_waits:
                    # sem_ge waits on the same semaphore are subsumed by the
                    # highest value: dedupe before splitting (the Tile drain
                    # attaches one wait per logical-processor tick, mostly on
                    # the same few engine sems).
                    best: dict = {}
                    order = []
                    passthrough = []
                    for w in si.on_wait:
                        if (
                            "ge" in str(w.wait_mode)
                            and w.wait_value is not None
                        ):
                            key = (str(w.sync_type), w.id)
                            cur = best.get(key)
                            if cur is None:
                                order.append(key)
                                best[key] = w
                            elif w.wait_value > cur.wait_value:
                                best[key] = w
                        else:
                            passthrough.append(w)
                    waits = passthrough + [best[k] for k in order]
                    if len(waits) <= max_waits:
                        inst.sync_info = mybir.SyncInfo(
                            on_wait=waits, on_update=list(si.on_update or [])
                        )
                        out.append(inst)
                        changed = True
                        continue
                    extra, keep = waits[:-max_waits], waits[-max_waits:]
                    while extra:
                        chunk, extra = extra[:max_waits], extra[max_waits:]
                        n += 1
                        out.append(
                            mybir.InstNoOp(
                                name=f"waitsplit-{n}",
                                engine=inst.engine,
                                sync_info=mybir.SyncInfo(on_wait=chunk, on_update=[]),
                            )
                        )
                    inst.sync_info = mybir.SyncInfo(
                        on_wait=keep, on_update=list(si.on_update or [])
                    )
                    changed = True
                out.append(inst)
            if changed:
                bb.instructions = out
    return n


def _tt_max(nc, eng, out_ap, in0, in1):
    """tensor_tensor(max) emitted on an arbitrary engine proxy."""
    return eng.add_instruction(
        mybir.InstTensorTensor(
            name=nc.get_next_instruction_name(),
            op=mybir.AluOpType.max,
            ins=[eng.lower_ap(in0), eng.lower_ap(in1)],
            outs=[eng.lower_ap(out_ap)],
        )
    )


def build_program() -> bass.Bass:
    nc = bass.Bass("TRN2", target_bir_lowering=True, debug=False)
    tmplA = nc.declare_dram_parameter("tmplA", [K, HALF], BF16, isOutput=False)
    srcA = nc.declare_dram_parameter("srcA", [K, M], BF16, isOutput=False)
    ident = nc.declare_dram_parameter("ident", [128, 128], F16, isOutput=False)
    negd01 = nc.declare_dram_parameter("negd01", [128, TBLOCKS], F32, isOutput=True)
    negd10 = nc.declare_dram_parameter("negd10", [128, M // 128], F32, isOutput=True)

    with _ChunkedDrainTileContext(nc) as tc:
        with (
            tc.tile_pool(name="inp", bufs=1) as inp,
            tc.tile_pool(name="psum", bufs=2, space="PSUM") as pp,
            tc.tile_pool(name="ct", bufs=4) as ctp,
            tc.tile_pool(name="s1", bufs=2) as s1p,
            tc.tile_pool(name="tree", bufs=2) as treep,
            tc.tile_pool(name="qs", bufs=2) as qsp,
            tc.tile_pool(name="outp", bufs=1) as outp,
        ):
            # input DMAs on separate queues; template block 0 lands first so
            # the first matmul can start immediately.
            tmpl_sb = inp.tile([K, HALF], BF16)
            nc.sync.dma_start(tmpl_sb[:, 0:128], tmplA[:, 0:128])
            nc.sync.dma_start(tmpl_sb[:, 128:HALF], tmplA[:, 128:HALF])
            src_sb = inp.tile([K, M], BF16)
            for piece in range(4):
                nc.gpsimd.dma_start(
                    src_sb[:, bass.ts(piece, M // 4)], srcA[:, bass.ts(piece, M // 4)]
                )
            id_sb = inp.tile([128, 128], F16)
            nc.gpsimd.dma_start(id_sb[:], ident[:])

            d01sb = outp.tile([128, TBLOCKS], F32)
            d10sb = outp.tile([128, M // 128], F32)

            acc = None
            P_prev = None
            s1 = None
            for i in range(TBLOCKS):
                # ---- PE: 8 matmuls -> 2 psum tiles
                ps_h = []
                for h in range(2):
                    ps = pp.tile([128, SFREE], F32, tag="ps")
                    for jj in range(SFREE // 512):
                        nc.tensor.matmul(
                            ps[:, bass.ts(jj, 512)],
                            lhsT=tmpl_sb[:, bass.ts(i, 128)],
                            rhs=src_sb[:, h * SFREE + jj * 512 : h * SFREE + (jj + 1) * 512],
                            start=True,
                            stop=True,
                        )
                    ps_h.append(ps)

                # ---- ACT: cast to f16 (loop pacer)
                ct = ctp.tile([128, M], F16, tag="ct")
                nc.scalar.copy(ct[:, 0:SFREE], ps_h[0][:])
                nc.scalar.copy(ct[:, SFREE:M], ps_h[1][:])

                # ---- DVE d01 fold1 into pair staging [128, 2, 2048]
                # (for the last block this is emitted after the d10 chain so
                # the transpose tail overlaps the remaining d01 closure)
                if i % 2 == 0:
                    s1 = s1p.tile([128, 2 * SFREE], F16, tag="s1")
                if i < TBLOCKS - 1:
                    nc.vector.tensor_tensor(
                        s1[:, bass.ts(i % 2, SFREE)], ct[:, 0:SFREE], ct[:, SFREE:M],
                        op=MAX,
                    )

                # ---- DVE d10: pair tree level + running acc
                if i % 2 == 0:
                    ct_even = ct
                else:
                    P = treep.tile([128, M], F16, tag="P")
                    nc.vector.tensor_tensor(
                        P[:].rearrange("p (b c) -> p b c", c=SFREE),
                        ct_even[:].rearrange("p (b c) -> p b c", c=SFREE),
                        ct[:].rearrange("p (b c) -> p b c", c=SFREE),
                        op=MAX,
                    )
                    if P_prev is None:
                        P_prev = P
                    elif i < TBLOCKS - 1:
                        acc_new = treep.tile([128, M], F16, tag="acc")
                        nc.vector.tensor_tensor(
                            acc_new[:].rearrange("p (b c) -> p b c", c=SFREE),
                            (acc if acc is not None else P_prev)[:].rearrange(
                                "p (b c) -> p b c", c=SFREE
                            ),
                            P[:].rearrange("p (b c) -> p b c", c=SFREE),
                            op=MAX,
                        )
                        acc = acc_new

                # ---- last block: d10 tail first (accq quarters + PE
                # transposes + TR), then the deferred fold1 + closure
                if i == TBLOCKS - 1:
                    accq = treep.tile([128, M], F16, tag="acc")
                    for qq in range(4):
                        csl = slice(qq * 1024, (qq + 1) * 1024)
                        nc.vector.tensor_tensor(
                            accq[:, csl], acc[:, csl], P[:, csl], op=MAX
                        )
                        psT = pp.tile([128, 1024], F16, tag="ps")
                        for t in range(8):
                            nc.tensor.transpose(
                                psT[:, bass.ts(t, 128)],
                                accq[:, qq * 1024 + t * 128 : qq * 1024 + (t + 1) * 128],
                                id_sb[:],
                            )
                        nc.vector.tensor_reduce(
                            d10sb[:, qq * 8 : (qq + 1) * 8],
                            psT[:].rearrange("p (t c) -> p t c", c=128),
                            axis=mybir.AxisListType.X,
                            op=MAX,
                        )
                        nc.sync.dma_start(
                            negd10[:, qq * 8 : (qq + 1) * 8],
                            d10sb[:, qq * 8 : (qq + 1) * 8],
                        )
                    nc.vector.tensor_tensor(
                        s1[:, bass.ts(i % 2, SFREE)], ct[:, 0:SFREE], ct[:, SFREE:M],
                        op=MAX,
                    )

                # ---- DVE d01 pair-group fold + TR -> negd01 slots
                if i % 2 == 1:
                    g = i // 2
                    sv = s1[:].rearrange("p (b c) -> p b c", c=SFREE)
                    g2 = qsp.tile([128, 2 * (SFREE // 2)], F16, tag="g2")
                    nc.vector.tensor_tensor(
                        g2[:].rearrange("p (b c) -> p b c", c=SFREE // 2),
                        sv[:, :, 0 : SFREE // 2],
                        sv[:, :, SFREE // 2 : SFREE],
                        op=MAX,
                    )
                    g3 = qsp.tile([128, 2 * (SFREE // 4)], F16, tag="g3")
                    nc.vector.tensor_tensor(
                        g3[:].rearrange("p (b c) -> p b c", c=SFREE // 4),
                        g2[:].rearrange("p (b c) -> p b c", c=SFREE // 2)[
                            :, :, 0 : SFREE // 4
                        ],
                        g2[:].rearrange("p (b c) -> p b c", c=SFREE // 2)[
                            :, :, SFREE // 4 : SFREE // 2
                        ],
                        op=MAX,
                    )
                    g4 = qsp.tile([128, 2 * (SFREE // 8)], F16, tag="g4")
                    nc.vector.tensor_tensor(
                        g4[:].rearrange("p (b c) -> p b c", c=SFREE // 8),
                        g3[:].rearrange("p (b c) -> p b c", c=SFREE // 4)[
                            :, :, 0 : SFREE // 8
                        ],
                        g3[:].rearrange("p (b c) -> p b c", c=SFREE // 4)[
                            :, :, SFREE // 8 : SFREE // 4
                        ],
                        op=MAX,
                    )
                    g5 = qsp.tile([128, 2 * (SFREE // 16)], F16, tag="g5")
                    nc.vector.tensor_tensor(
                        g5[:].rearrange("p (b c) -> p b c", c=SFREE // 16),
                        g4[:].rearrange("p (b c) -> p b c", c=SFREE // 8)[
                            :, :, 0 : SFREE // 16
                        ],
                        g4[:].rearrange("p (b c) -> p b c", c=SFREE // 8)[
                            :, :, SFREE // 16 : SFREE // 8
                        ],
                        op=MAX,
                    )
                    nc.vector.tensor_reduce(
                        d01sb[:, 2 * g : 2 * g + 2],
                        g5[:].rearrange("p (b c) -> p b c", c=SFREE // 16),
                        axis=mybir.AxisListType.X,
                        op=MAX,
                    )

            nc.sync.dma_start(negd01[:], d01sb[:])
    _split_multi_waits(nc)
    return nc


_PROGRAM = None


def get_program() -> bass.Bass:
    global _PROGRAM
    if _PROGRAM is None:
        _PROGRAM = build_program()
    return _PROGRAM


def _split3(x: np.ndarray):
    bf = ml_dtypes.bfloat16
    h1 = x.astype(bf).astype(np.float32)
    h2 = (x - h1).astype(bf).astype(np.float32)
    h3 = (x - h1 - h2).astype(bf).astype(np.float32)
    return h1, h2, h3


# cross-product levels kept: everything with combined magnitude >= ~2^-27
_PAIRS = [(0, 0), (0, 1), (1, 0), (0, 2), (1, 1), (2, 0)]


def make_in_maps(template: np.ndarray, source: np.ndarray) -> list[dict]:
    """Host-side prep: split-bf16 augmented K=24 representations, sharded per
    core. Core c -> batch c//2, template half c%2."""
    template = np.asarray(template, dtype=np.float32)
    source = np.asarray(source, dtype=np.float32)
    bf = ml_dtypes.bfloat16
    in_maps = []
    for c in range(N_CORES):
        b, hh = divmod(c, 2)
        t = template[b, hh * HALF : (hh + 1) * HALF]  # [HALF, 3]
        s = source[b]  # [M, 3]
        T = _split3(t)
        U = _split3((2.0 * s).astype(np.float32))
        nt = (t.astype(np.float64) ** 2).sum(-1).astype(np.float32)
        ns = (s.astype(np.float64) ** 2).sum(-1).astype(np.float32)
        NT = _split3(nt)
        NS = _split3(ns)
        ones_t = np.ones_like(nt)
        ones_s = np.ones_like(ns)
        a_rows, b_rows = [], []
        for cc in range(3):
            for (ii, jj) in _PAIRS:
                a_rows.append(T[ii][:, cc])
                b_rows.append(U[jj][:, cc])
        for kk in range(3):
            a_rows.append(-NT[kk])
            b_rows.append(ones_s)
            a_rows.append(-ones_t)
            b_rows.append(NS[kk])
        tmplA = np.stack(a_rows, 0).astype(bf)  # [K, HALF]
        srcA = np.stack(b_rows, 0).astype(bf)  # [K, M]
        in_maps.append(
            {
                "tmplA": np.ascontiguousarray(tmplA),
                "srcA": np.ascontiguousarray(srcA),
                "ident": np.eye(128, dtype=np.float16),
            }
        )
    return in_maps


def combine(results: list[dict]) -> np.ndarray:
    """Gather per-core partials into the scalar loss (float64 accumulation)."""
    per_batch = []
    for b in range(B):
        r0, r1 = results[2 * b], results[2 * b + 1]
        d01_parts = []
        for r in (r0, r1):
            nd01 = r["negd01"].astype(np.float64)  # [128, 16]
            # template index within half = i*128 + p -> transpose to [16,128]
            d01_parts.append(nd01.T.reshape(-1))
        d01 = -np.concatenate(d01_parts)  # [4096]
        # negd10[n_loc, h*16+t] for source index h*2048 + t*128 + n_loc
        nd10 = np.maximum(
            r0["negd10"].astype(np.float64), r1["negd10"].astype(np.float64)
        )
        d10 = -nd10.T.reshape(-1)  # [32,128] -> index t'*128+n_loc with t'=h*16+t
        per_batch.append(d01.mean() + d10.mean())
    return np.asarray(np.mean(per_batch), dtype=np.float32)


def _axon_reset():
    """Recover a wedged NeuronCore (NRT_EXEC_UNIT_UNRECOVERABLE) left by a
    previous crashed run, via the axon sidechannel."""
    try:
        import ctypes

        import jax

        jax.devices()
        lib = ctypes.CDLL("/opt/axon/libaxon_pjrt.so")
        lib.axon_reset.restype = ctypes.c_int64
        lib.axon_reset()
    except Exception:
        pass


def kernel(template: np.ndarray, source: np.ndarray) -> np.ndarray:
    nc = get_program()
    in_maps = make_in_maps(template, source)
    try:
        res = run_bass_kernel_spmd(nc, in_maps, list(range(N_CORES)))
    except Exception:
        _axon_reset()
        res = run_bass_kernel_spmd(nc, in_maps, list(range(N_CORES)))
    return combine(res.results)



# revision 12
# speedup vs baseline: 1.2103x; 1.0034x over previous
"""Chamfer distance loss on 8 Trainium2 NeuronCores.

Problem: template/source [4, 4096, 3] f32 -> scalar loss
  d[b,n,m] = ||t_n - s_m||^2 ; mean_n(min_m d) + mean_m(min_n d), mean over b.

Strategy (data-parallel over batch x template-half, 2 cores per batch):
  Each core handles one batch's full source set (4096 pts) against one half of
  the template set (2048 pts). Distances come from a single matmul in NEGATED
  split-bf16 form (K=24 rows reproduce negd = 2 t.s - |t|^2 - |s|^2 to
  near-fp32 accuracy at full PE rate), so every reduction is a MAX.

  v4 pipeline (110 -> ~89us on the harness metric):
  Per template block i (16 blocks of 128 points):
    - 8 matmuls -> 2 psum tiles [128, 2048] f32 (2-tile rotation).
    - ACT casts both tiles -> ct_i [128, 4096] f16. ACT is the loop pacer
      (~63us of ACTIVATE); PE (~57us) and DVE (~83us busy) overlap under it.
    - DVE d01 (row max): fold1 ct_i -> pair staging; per pair of blocks a
      4-level fold chain + small TENSOR_REDUCE -> negd01 slots. All TT ops
      are f16 SBUF->SBUF, 2x DVE mode, sized >=256 elems.
    - DVE d10 (col max): P_j = max(ct_2j, ct_2j+1), then a running
      acc = max(acc, P_j). 15 ops of [128, 4096] at 2x.
  Tail: at the last block the d10 chain runs FIRST (acc quarters -> 8 PE
  transposes -> psum f16 -> TENSOR_REDUCE -> negd10 slots per quarter), then
  the deferred d01 fold1+closure overlaps the transpose/reduce pipeline. Engine-op constraints found the
  hard way: TensorTensor/TensorReduce only lower on DVE (walrus rejects them
  on ACT/Pool), TT reads at most one PSUM operand, DMA cannot touch PSUM and
  its compute mode is add-only, matmul PSUM output must be f32.
"""

import numpy as np
import ml_dtypes

import concourse.bass as bass
import concourse.bass_utils as bass_utils
import concourse.tile as tile
from concourse import mybir
from concourse.bass_utils import run_bass_kernel_spmd
from concourse.vector_clock import ScopedClock

B, N, M = 4, 4096, 4096
HALF = N // 2  # template half per core: 2048
N_CORES = 8
TBLOCKS = HALF // 128  # 16 template blocks
SFREE = M // 2  # source half width: 2048
K = 24

F32 = mybir.dt.float32
F16 = mybir.dt.float16
BF16 = mybir.dt.bfloat16
MAX = mybir.AluOpType.max

_MAX_DRAIN_WAITS = 1

# pairs whose running-max op runs on the Activation engine (ACT has slack;
# DVE is the pacer). Empty tuple = all on DVE.
ACT_ACC_PAIRS = ()


class _ChunkedDrainTileContext(tile.TileContext):
    """The walrus build used by the axon/PJRT path rejects instructions with
    more than a couple of sync waits; Tile's exit drain attaches one wait per
    live logical processor. Split them across sequential drains."""

    def _drain_and_barrier(self, tick_clock, wait_clock):
        # Stock Tile emits drain + two all-engine barriers around semaphore
        # clears (~9us of measured tail). The kernel PREAMBLE already clears
        # semaphore ranges 150..255 on every execution, so end-of-kernel
        # clears are redundant for re-runs; the only load-bearing waits are
        # the DMA-queue completion sems (output data must land before the
        # program is considered done). Keep just those, on the sync engine.
        drain_inst = self.nc.sync.drain()
        wait_clock.add_sem_waits(
            drain_inst.ins, ScopedClock({None: tick_clock.global_clock})
        )
        si = drain_inst.ins.sync_info
        waits = list(si.on_wait) if si is not None and si.on_wait else []
        dma_waits = [w for w in waits if w.ant_name and "DMA" in w.ant_name]
        keep = dma_waits if dma_waits else waits
        drain_inst.ins.sync_info = mybir.SyncInfo(
            on_wait=keep[:_MAX_DRAIN_WAITS],
            on_update=list(si.on_update or []) if si else [],
        )
        rest = keep[_MAX_DRAIN_WAITS:]
        while rest:
            d = self.nc.sync.drain()
            d.ins.sync_info = mybir.SyncInfo(
                on_wait=rest[:_MAX_DRAIN_WAITS], on_update=[]
            )
            rest = rest[_MAX_DRAIN_WAITS:]

        assert self.sems is not None
        popped = self.nc._tile_sem_poison_stack.pop()
        assert popped is self._sem_poison


def _split_multi_waits(nc: bass.Bass, max_waits: int = 1) -> int:
    """This walrus build rejects instructions carrying several sync waits.
    Hoist excess waits onto NoOps inserted before the offender on the same
    engine — same-engine program order preserves the semantics."""
    n = 0
    for f in nc.m.functions:
        for bb in f.blocks:
            insts = list(bb.instructions)
            out = []
            changed = False
            for inst in insts:
                si = inst.sync_info
                if si is not None and si.on_wait and len(si.on_wait) > max_waits:
                    # sem_ge waits on the same semaphore are subsumed by the
                    # highest value: dedupe before splitting (the Tile drain
                    # attaches one wait per logical-processor tick, mostly on
                    # the same few engine sems).
                    best: dict = {}
                    order = []
                    passthrough = []
                    for w in si.on_wait:
                        if (
                            "ge" in str(w.wait_mode)
                            and w.wait_value is not None
                        ):
                            key = (str(w.sync_type), w.id)
                            cur = best.get(key)
                            if cur is None:
                                order.append(key)
                                best[key] = w
                            elif w.wait_value > cur.wait_value:
                                best[key] = w
                        else:
                            passthrough.append(w)
                    waits = passthrough + [best[k] for k in order]
                    if len(waits) <= max_waits:
                        inst.sync_info = mybir.SyncInfo(
                            on_wait=waits, on_update=list(si.on_update or [])
                        )
                        out.append(inst)
                        changed = True
                        continue
                    extra, keep = waits[:-max_waits], waits[-max_waits:]
                    while extra:
                        chunk, extra = extra[:max_waits], extra[max_waits:]
                        n += 1
                        out.append(
                            mybir.InstNoOp(
                                name=f"waitsplit-{n}",
                                engine=inst.engine,
                                sync_info=mybir.SyncInfo(on_wait=chunk, on_update=[]),
                            )
                        )
                    inst.sync_info = mybir.SyncInfo(
                        on_wait=keep, on_update=list(si.on_update or [])
                    )
                    changed = True
                out.append(inst)
            if changed:
                bb.instructions = out
    return n


def _tt_max(nc, eng, out_ap, in0, in1):
    """tensor_tensor(max) emitted on an arbitrary engine proxy."""
    return eng.add_instruction(
        mybir.InstTensorTensor(
            name=nc.get_next_instruction_name(),
            op=mybir.AluOpType.max,
            ins=[eng.lower_ap(in0), eng.lower_ap(in1)],
            outs=[eng.lower_ap(out_ap)],
        )
    )


def build_program() -> bass.Bass:
    nc = bass.Bass("TRN2", target_bir_lowering=True, debug=False)
    tmplA = nc.declare_dram_parameter("tmplA", [K, HALF], BF16, isOutput=False)
    srcA = nc.declare_dram_parameter("srcA", [K, M], BF16, isOutput=False)
    ident = nc.declare_dram_parameter("ident", [128, 128], F16, isOutput=False)
    negd01 = nc.declare_dram_parameter("negd01", [128, TBLOCKS], F32, isOutput=True)
    negd10 = nc.declare_dram_parameter("negd10", [128, M // 128], F32, isOutput=True)

    with _ChunkedDrainTileContext(nc) as tc:
        with (
            tc.tile_pool(name="inp", bufs=1) as inp,
            tc.tile_pool(name="psum", bufs=2, space="PSUM") as pp,
            tc.tile_pool(name="ct", bufs=4) as ctp,
            tc.tile_pool(name="s1", bufs=2) as s1p,
            tc.tile_pool(name="tree", bufs=2) as treep,
            tc.tile_pool(name="qs", bufs=2) as qsp,
            tc.tile_pool(name="outp", bufs=1) as outp,
        ):
            # input DMAs on separate queues; template block 0 lands first so
            # the first matmul can start immediately.
            tmpl_sb = inp.tile([K, HALF], BF16)
            nc.sync.dma_start(tmpl_sb[:, 0:128], tmplA[:, 0:128])
            nc.sync.dma_start(tmpl_sb[:, 128:HALF], tmplA[:, 128:HALF])
            src_sb = inp.tile([K, M], BF16)
            for piece in range(4):
                nc.gpsimd.dma_start(
                    src_sb[:, bass.ts(piece, M // 4)], srcA[:, bass.ts(piece, M // 4)]
                )
            id_sb = inp.tile([128, 128], F16)
            nc.gpsimd.dma_start(id_sb[:], ident[:])

            d01sb = outp.tile([128, TBLOCKS], F32)
            d10sb = outp.tile([128, M // 128], F32)

            acc = None
            P_prev = None
            s1 = None
            for i in range(TBLOCKS):
                # ---- PE: 8 matmuls -> 2 psum tiles
                ps_h = []
                for h in range(2):
                    ps = pp.tile([128, SFREE], F32, tag="ps")
                    for jj in range(SFREE // 512):
                        nc.tensor.matmul(
                            ps[:, bass.ts(jj, 512)],
                            lhsT=tmpl_sb[:, bass.ts(i, 128)],
                            rhs=src_sb[:, h * SFREE + jj * 512 : h * SFREE + (jj + 1) * 512],
                            start=True,
                            stop=True,
                        )
                    ps_h.append(ps)

                # ---- ACT: cast to f16 (loop pacer)
                ct = ctp.tile([128, M], F16, tag="ct")
                nc.scalar.copy(ct[:, 0:SFREE], ps_h[0][:])
                nc.scalar.copy(ct[:, SFREE:M], ps_h[1][:])

                # ---- DVE d01 fold1 into pair staging [128, 2, 2048]
                # (for the last block this is emitted after the d10 chain so
                # the transpose tail overlaps the remaining d01 closure)
                if i % 2 == 0:
                    s1 = s1p.tile([128, 2 * SFREE], F16, tag="s1")
                if i < TBLOCKS - 1:
                    nc.vector.tensor_tensor(
                        s1[:, bass.ts(i % 2, SFREE)], ct[:, 0:SFREE], ct[:, SFREE:M],
                        op=MAX,
                    )

                # ---- DVE d10: pair tree level + running acc
                if i % 2 == 0:
                    ct_even = ct
                else:
                    P = treep.tile([128, M], F16, tag="P")
                    nc.vector.tensor_tensor(
                        P[:].rearrange("p (b c) -> p b c", c=SFREE),
                        ct_even[:].rearrange("p (b c) -> p b c", c=SFREE),
                        ct[:].rearrange("p (b c) -> p b c", c=SFREE),
                        op=MAX,
                    )
                    if P_prev is None:
                        P_prev = P
                    elif i < TBLOCKS - 1:
                        acc_new = treep.tile([128, M], F16, tag="acc")
                        nc.vector.tensor_tensor(
                            acc_new[:].rearrange("p (b c) -> p b c", c=SFREE),
                            (acc if acc is not None else P_prev)[:].rearrange(
                                "p (b c) -> p b c", c=SFREE
                            ),
                            P[:].rearrange("p (b c) -> p b c", c=SFREE),
                            op=MAX,
                        )
                        acc = acc_new

                # ---- last block: d10 tail first (accq quarters + PE
                # transposes + TR), then the deferred fold1 + closure
                if i == TBLOCKS - 1:
                    accq = treep.tile([128, M], F16, tag="acc")
                    for qq in range(4):
                        csl = slice(qq * 1024, (qq + 1) * 1024)
                        nc.vector.tensor_tensor(
                            accq[:, csl], acc[:, csl], P[:, csl], op=MAX
                        )
                        psT = pp.tile([128, 1024], F16, tag="ps")
                        for t in range(8):
                            nc.tensor.transpose(
                                psT[:, bass.ts(t, 128)],
                                accq[:, qq * 1024 + t * 128 : qq * 1024 + (t + 1) * 128],
                                id_sb[:],
                            )
                        nc.vector.tensor_reduce(
                            d10sb[:, qq * 8 : (qq + 1) * 8],
                            psT[:].rearrange("p (t c) -> p t c", c=128),
                            axis=mybir.AxisListType.X,
                            op=MAX,
                        )
                        nc.sync.dma_start(
                            negd10[:, qq * 8 : (qq + 1) * 8],
                            d10sb[:, qq * 8 : (qq + 1) * 8],
                        )
                    nc.vector.tensor_tensor(
                        s1[:, bass.ts(i % 2, SFREE)], ct[:, 0:SFREE], ct[:, SFREE:M],
                        op=MAX,
                    )

                # ---- DVE d01 pair-group fold + TR -> negd01 slots
                if i % 2 == 1:
                    g = i // 2
                    sv = s1[:].rearrange("p (b c) -> p b c", c=SFREE)
                    g2 = qsp.tile([128, 2 * (SFREE // 2)], F16, tag="g2")
                    nc.vector.tensor_tensor(
                        g2[:].rearrange("p (b c) -> p b c", c=SFREE // 2),
                        sv[:, :, 0 : SFREE // 2],
                        sv[:, :, SFREE // 2 : SFREE],
                        op=MAX,
                    )
                    g3 = qsp.tile([128, 2 * (SFREE // 4)], F16, tag="g3")
                    nc.vector.tensor_tensor(
                        g3[:].rearrange("p (b c) -> p b c", c=SFREE // 4),
                        g2[:].rearrange("p (b c) -> p b c", c=SFREE // 2)[
                            :, :, 0 : SFREE // 4
                        ],
                        g2[:].rearrange("p (b c) -> p b c", c=SFREE // 2)[
                            :, :, SFREE // 4 : SFREE // 2
                        ],
                        op=MAX,
                    )
                    g4 = qsp.tile([128, 2 * (SFREE // 8)], F16, tag="g4")
                    nc.vector.tensor_tensor(
                        g4[:].rearrange("p (b c) -> p b c", c=SFREE // 8),
                        g3[:].rearrange("p (b c) -> p b c", c=SFREE // 4)[
                            :, :, 0 : SFREE // 8
                        ],
                        g3[:].rearrange("p (b c) -> p b c", c=SFREE // 4)[
                            :, :, SFREE // 8 : SFREE // 4
                        ],
                        op=MAX,
                    )
                    g5 = qsp.tile([128, 2 * (SFREE // 16)], F16, tag="g5")
                    nc.vector.tensor_tensor(
                        g5[:].rearrange("p (b c) -> p b c", c=SFREE // 16),
                        g4[:].rearrange("p (b c) -> p b c", c=SFREE // 8)[
                            :, :, 0 : SFREE // 16
                        ],
                        g4[:].rearrange("p (b c) -> p b c", c=SFREE // 8)[
                            :, :, SFREE // 16 : SFREE // 8
                        ],
                        op=MAX,
                    )
                    nc.vector.tensor_reduce(
                        d01sb[:, 2 * g : 2 * g + 2],
                        g5[:].rearrange("p (b c) -> p b c", c=SFREE // 16),
                        axis=mybir.AxisListType.X,
                        op=MAX,
                    )

            nc.sync.dma_start(negd01[:], d01sb[:])
    _split_multi_waits(nc)
    return nc


_PROGRAM = None


def get_program() -> bass.Bass:
    global _PROGRAM
    if _PROGRAM is None:
        _PROGRAM = build_program()
    return _PROGRAM


def _split3(x: np.ndarray):
    bf = ml_dtypes.bfloat16
    h1 = x.astype(bf).astype(np.float32)
    h2 = (x - h1).astype(bf).astype(np.float32)
    h3 = (x - h1 - h2).astype(bf).astype(np.float32)
    return h1, h2, h3


# cross-product levels kept: everything with combined magnitude >= ~2^-27
_PAIRS = [(0, 0), (0, 1), (1, 0), (0, 2), (1, 1), (2, 0)]


def make_in_maps(template: np.ndarray, source: np.ndarray) -> list[dict]:
    """Host-side prep: split-bf16 augmented K=24 representations, sharded per
    core. Core c -> batch c//2, template half c%2."""
    template = np.asarray(template, dtype=np.float32)
    source = np.asarray(source, dtype=np.float32)
    bf = ml_dtypes.bfloat16
    in_maps = []
    for c in range(N_CORES):
        b, hh = divmod(c, 2)
        t = template[b, hh * HALF : (hh + 1) * HALF]  # [HALF, 3]
        s = source[b]  # [M, 3]
        T = _split3(t)
        U = _split3((2.0 * s).astype(np.float32))
        nt = (t.astype(np.float64) ** 2).sum(-1).astype(np.float32)
        ns = (s.astype(np.float64) ** 2).sum(-1).astype(np.float32)
        NT = _split3(nt)
        NS = _split3(ns)
        ones_t = np.ones_like(nt)
        ones_s = np.ones_like(ns)
        a_rows, b_rows = [], []
        for cc in range(3):
            for (ii, jj) in _PAIRS:
                a_rows.append(T[ii][:, cc])
                b_rows.append(U[jj][:, cc])
        for kk in range(3):
            a_rows.append(-NT[kk])
            b_rows.append(ones_s)
            a_rows.append(-ones_t)
            b_rows.append(NS[kk])
        tmplA = np.stack(a_rows, 0).astype(bf)  # [K, HALF]
        srcA = np.stack(b_rows, 0).astype(bf)  # [K, M]
        in_maps.append(
            {
                "tmplA": np.ascontiguousarray(tmplA),
                "srcA": np.ascontiguousarray(srcA),
                "ident": np.eye(128, dtype=np.float16),
            }
        )
    return in_maps


def combine(results: list[dict]) -> np.ndarray:
    """Gather per-core partials into the scalar loss (float64 accumulation)."""
    per_batch = []
    for b in range(B):
        r0, r1 = results[2 * b], results[2 * b + 1]
        d01_parts = []
        for r in (r0, r1):
            nd01 = r["negd01"].astype(np.float64)  # [128, 16]
            # template index within half = i*128 + p -> transpose to [16,128]
            d01_parts.append(nd01.T.reshape(-1))
        d01 = -np.concatenate(d01_parts)  # [4096]
        # negd10[n_loc, h*16+t] for source index h*2048 + t*128 + n_loc
        nd10 = np.maximum(
            r0["negd10"].astype(np.float64), r1["negd10"].astype(np.float64)
        )
        d10 = -nd10.T.reshape(-1)  # [32,128] -> index t'*128+n_loc with t'=h*16+t
        per_batch.append(d01.mean() + d10.mean())
    return np.asarray(np.mean(per_batch), dtype=np.float32)


def _axon_reset():
    """Recover a wedged NeuronCore (NRT_EXEC_UNIT_UNRECOVERABLE) left by a
    previous crashed run, via the axon sidechannel."""
    try:
        import ctypes

        import jax

        jax.devices()
        lib = ctypes.CDLL("/opt/axon/libaxon_pjrt.so")
        lib.axon_reset.restype = ctypes.c_int64
        lib.axon_reset()
    except Exception:
        pass


def kernel(template: np.ndarray, source: np.ndarray) -> np.ndarray:
    nc = get_program()
    in_maps = make_in_maps(template, source)
    try:
        res = run_bass_kernel_spmd(nc, in_maps, list(range(N_CORES)))
    except Exception:
        _axon_reset()
        res = run_bass_kernel_spmd(nc, in_maps, list(range(N_CORES)))
    return combine(res.results)

